# revision 19
# baseline (speedup 1.0000x reference)
"""Trainium2 Bass kernel for nn_DeconvLayer (causal IIR filter).

Math: the reference IIR v[i] = x[i] + sum_j w[j] v[i-1-j] (i >= F, else 0)
has a geometrically-decaying impulse response h (|h[127]| ~ 3e-13), so it
equals a 128-tap causal FIR applied to x with the first F columns zeroed.

This kernel computes only the RESIDUAL c = y - x on device:

    c[:, n] = sum_{k=1}^{127} h[k] * xz[:, n-k]      (no identity tap)

as block-Toeplitz matmuls with A0'[t,i] = h[i-t] for i > t (diag zeroed)
and A1[t,i] = h[128+i-t] for t > i.  The host reconstructs y = x + c/8
with its exact fp32 copy of x, so the large identity term never
round-trips through low precision.

TensorEngine trick: the two Toeplitz matmuls (current block x A0' +
previous block x A1) fuse into ONE DoubleRow fp8 matmul with a 256-deep
contraction: the stationary holds [A1 | A0'] as a [128, 2, 128] pair AP
and the moving operand is an overlapping [128, 2, 512] AP over the x
tile (pair stride = one 256-column block).  This halves TensorE busy
time — fp8 runs at 2 MACs/cell/cycle only in DoubleRow mode.

Precision/traffic: x is sent as fp8 e4m3 (DoubleRow requires e4/e5) and
c returned as fp8 e3m4 scaled by 8 (folded into the stationaries), so
HBM traffic is 8.4 MB/core total.  PSUM accumulates in fp32; end-to-end
rel error ~6.8e-3 vs the 2e-2 gate.

Layout trick: the host uploads x transposed AND 128-blocked as
[t, chunk, r] so time lands on the partition axis with no on-device
transposes and every DMA partition-line is one contiguous read.

Sharding: N = 131072 split into 8 column slabs of 16384 (+128-step halo
from the left neighbor), all B = 256 rows on every core.
"""

import sys

import numpy as np

if "/opt/trn_rl_repo" not in sys.path:
    sys.path.insert(0, "/opt/trn_rl_repo")

B = 256
N = 131072
F = 8
K = 128          # FIR taps == block size
P = 128          # partitions / block size
NCORES = 8
CORE_COLS = N // NCORES       # 16384 time steps per core
NCHUNK = CORE_COLS // P       # 128 chunks per core
CPI = 32                      # chunks produced per iteration
NIT = NCHUNK // CPI           # 4 iterations per core
FREE = B                      # free dim per chunk (batch rows)
QG = CPI * FREE // 512        # 512-wide PSUM groups per iteration (16)
CSCALE = 8.0                  # residual output scale (folded into A)

_CACHE = {}


def _impulse_response(w64):
    h = np.zeros(K, np.float64)
    h[0] = 1.0
    for n in range(1, K):
        acc = 0.0
        for j in range(min(F, n)):
            acc += w64[j] * h[n - 1 - j]
        h[n] = acc
    return h


def _toeplitz_mats(h):
    """A0'[t, i] = h[i-t] for i > t (identity tap dropped);
    A1[t, i] = h[128+i-t] for t > i.  Returned in float64."""
    a0 = np.zeros((P, P), np.float64)
    a1 = np.zeros((P, P), np.float64)
    for t in range(P):
        for i in range(P):
            if i > t:
                a0[t, i] = h[i - t]
            elif t > i:
                a1[t, i] = h[K + i - t]
    return a0, a1


def _pair_moving_ap(xt, base):
    """Overlapping [128, 2, 512] AP over tile `xt`: pair 0 = cols
    [base, base+512) (previous block window), pair 1 = cols
    [base+256, base+768) (current block window)."""
    ap = xt[:, base : base + 768].rearrange("p (two n) -> p two n", two=2).copy()
    pat = ap.ap
    assert list(pat[1]) == [384, 2] and list(pat[2]) == [1, 384], pat
    ap.ap[1] = [256, 2]
    ap.ap[2] = [1, 512]
    assert list(ap.ap[1]) == [256, 2] and list(ap.ap[2]) == [1, 512], ap.ap
    return ap


def _build_nc():
    from contextlib import ExitStack

    import concourse.mybir as mybir
    import concourse.tile as tile
    from concourse import bacc

    f8i = mybir.dt.float8e4   # input / weights (DoubleRow needs e4/e5)
    f8o = mybir.dt.float8e3   # residual output

    nc = bacc.Bacc(
        "TRN2",
        target_bir_lowering=False,
        debug=False,
        enable_asserts=False,
        num_devices=NCORES,
    )
    # blocked transposed input: [t, chunk, r] flattened to [128, NCHUNK*FREE]
    W_IN = NCHUNK * FREE
    x_d = nc.dram_tensor("x8", [P, W_IN], f8i, kind="ExternalInput")
    # halo: previous core's last 128 steps (zeros for core 0)
    h_d = nc.dram_tensor("h8", [P, FREE], f8i, kind="ExternalInput")
    # fused stationary [A1 | A0'] side by side
    w_d = nc.dram_tensor("w2", [P, 2 * P], f8i, kind="ExternalInput")
    # blocked transposed residual output [t, chunk, r], fp8 e3m4, x8 scale
    c_out = nc.dram_tensor("c_out", [P, NCHUNK * FREE], f8o, kind="ExternalOutput")

    TW = CPI * FREE  # tile width (8192)

    with tile.TileContext(nc) as tc, ExitStack() as ctx:
        const = ctx.enter_context(tc.tile_pool(name="const", bufs=1))
        w2 = const.tile([P, 2 * P], f8i, tag="w2")
        nc.scalar.dma_start(w2[:], w_d[:, :])
        # pair view: [:, 0, :] = A1, [:, 1, :] = A0'
        w2_pair = w2[:].rearrange("p (two m) -> p two m", two=2)

        xpool = ctx.enter_context(tc.tile_pool(name="x", bufs=3))
        ypool = ctx.enter_context(tc.tile_pool(name="y", bufs=3))
        pspool = ctx.enter_context(tc.tile_pool(name="ps", bufs=4, space="PSUM"))

        DR = mybir.MatmulPerfMode.DoubleRow

        prev = None
        for it in range(NIT):
            u0 = it * TW
            # tile carries a leading halo chunk: [halo(256) | 32 chunks(8192)]
            xt = xpool.tile([P, FREE + TW], f8i)
            if it == 0:
                nc.sync.dma_start(xt[:, :FREE], h_d[:, :])
            else:
                # halo = previous tile's last chunk, copied within SBUF on the
                # otherwise-idle GpSimd engine
                nc.gpsimd.tensor_copy(xt[:, :FREE], prev[:, TW : TW + FREE])
            if it == 0:
                # small lead chunk so the first matmuls start ASAP, then the
                # rest in two halves
                L = 512
                nc.sync.dma_start(xt[:, FREE : FREE + L], x_d[:, u0 : u0 + L])
                H = (TW - L) // 2
                nc.sync.dma_start(
                    xt[:, FREE + L : FREE + L + H], x_d[:, u0 + L : u0 + L + H]
                )
                nc.sync.dma_start(xt[:, FREE + L + H :], x_d[:, u0 + L + H : u0 + TW])
            else:
                H = TW // 2
                nc.sync.dma_start(xt[:, FREE : FREE + H], x_d[:, u0 : u0 + H])
                nc.sync.dma_start(xt[:, FREE + H :], x_d[:, u0 + H : u0 + TW])
            prev = xt

            ybuf = ypool.tile([P, TW], f8o)
            # PSUM tiles span 2 banks (1024 fp32) so each drain instruction
            # amortizes its fixed cost over twice the data
            NPAIR = QG // 2
            pss = [
                pspool.tile([P, 1024], mybir.dt.float32, name=f"ps_{it}_{p}", tag="ps")
                for p in range(NPAIR)
            ]
            for p in range(NPAIR):
                for half in range(2):
                    q = 2 * p + half
                    # one DoubleRow matmul fuses the A1 (prev block) and A0'
                    # (current block) contributions: moving pair base is one
                    # block (256 cols) before this group's data
                    nc.tensor.matmul(
                        pss[p][:, half * 512 : (half + 1) * 512],
                        w2_pair,
                        _pair_moving_ap(xt, q * 512),
                        start=True,
                        stop=True,
                        perf_mode=DR,
                    )
                # drain the bank pair right away, evenly split across the two
                # PSUM-capable engines (different banks, so no collision)
                c0 = 2 * p * 512
                if p % 2 == 0:
                    nc.vector.tensor_copy(ybuf[:, c0 : c0 + 1024], pss[p][:])
                else:
                    nc.scalar.copy(ybuf[:, c0 : c0 + 1024], pss[p][:])
                # batch output DMAs, issued from the idle GpSimd sequencer
                # (SWDGE) to keep ACT free for PSUM drains.  The final
                # iteration flushes per pair so the drain overlaps the last
                # copies instead of trailing them.
                flush = 1 if it == NIT - 1 else 4
                if (p + 1) % flush == 0:
                    h0 = (p + 1 - flush) * 1024
                    nc.gpsimd.dma_start(
                        c_out[:, u0 + h0 : u0 + h0 + flush * 1024],
                        ybuf[:, h0 : h0 + flush * 1024],
                    )
    nc.compile()
    return nc


def _get_nc():
    if "nc" not in _CACHE:
        _CACHE["nc"] = _build_nc()
    return _CACHE["nc"]


LAST_RESULTS = None


def kernel(x, w=None, _trace=False, **_ignored):
    global LAST_RESULTS
    import ml_dtypes

    from concourse.bass_utils import run_bass_kernel_spmd

    f8i = ml_dtypes.float8_e4m3
    f8o = ml_dtypes.float8_e3m4

    x = np.asarray(x, dtype=np.float32)
    assert x.shape == (B, N)
    if w is None:
        import jax
        import jax.numpy as jnp

        key = jax.random.key(0)
        _, k2 = jax.random.split(key)
        w = np.asarray(jax.random.normal(k2, (F,), dtype=jnp.float32) * 0.05)
    w = np.asarray(w, dtype=np.float32)

    h = _impulse_response(w.astype(np.float64))
    a0, a1 = _toeplitz_mats(h)
    # fused stationary: [A1 | A0'], output scale folded in
    w2 = np.concatenate([a1 * CSCALE, a0 * CSCALE], axis=1).astype(f8i)

    # transposed, 128-blocked input: [t, chunk, r]
    xt = np.array(x.T)  # [N, B]
    xt[:F] = 0.0  # v[i] = 0 for i < F
    xb = np.ascontiguousarray(
        xt.reshape(NCORES * NCHUNK, P, B).transpose(1, 0, 2)
    )  # [128, 1024, 256] fp32
    xb_8 = xb.astype(f8i)
    zhalo = np.zeros((P, B), f8i)

    in_maps = []
    for c in range(NCORES):
        lo_c = c * NCHUNK
        sl = np.s_[:, lo_c : lo_c + NCHUNK, :]
        in_maps.append(
            {
                "x8": np.ascontiguousarray(xb_8[sl]).reshape(P, -1),
                "h8": zhalo if c == 0 else np.ascontiguousarray(xb_8[:, lo_c - 1, :]),
                "w2": w2,
            }
        )

    nc = _get_nc()
    res = run_bass_kernel_spmd(
        nc, in_maps, core_ids=list(range(NCORES)), trace=_trace
    )
    LAST_RESULTS = res
    # reassemble residual: per core [128, NCHUNK, FREE] -> [NCHUNK*P, FREE]
    parts = []
    for r in res.results:
        cb = r["c_out"].reshape(P, NCHUNK, B).transpose(1, 0, 2)  # [chunk, t, r]
        parts.append(cb.reshape(CORE_COLS, B))
    ct = np.concatenate(parts, axis=0).astype(np.float32)  # [N, B], c*8
    y = x + ct.T * np.float32(1.0 / CSCALE)
    y[:, :F] = 0.0  # v[i] = 0 for i < F (identity tap does not pass through)
    return y


if __name__ == "__main__":
    rng = np.random.default_rng(0)
    x = rng.standard_normal((B, N), dtype=np.float32)
    w = (rng.standard_normal(F) * 0.05).astype(np.float32)
    y = kernel(x, w)
    print("kernel ran, y shape:", y.shape)
